# revision 1
# baseline (speedup 1.0000x reference)
import sys

sys.path.insert(0, "/opt/trn_rl_repo")

from contextlib import ExitStack

import ml_dtypes
import numpy as np

from concourse import bass, mybir, tile
from concourse.bass_utils import run_bass_kernel_spmd
from concourse.vector_clock import ScopedClock


def _patched_drain_and_barrier(self, tick_clock, wait_clock):
    # Workaround: this compiler rejects a drain carrying >1 sem wait
    # ([NCC_INLA001]); split extra waits onto single-wait nops.
    drain_inst = self.nc.sync.drain()
    wait_clock.add_sem_waits(
        drain_inst.ins, ScopedClock({None: tick_clock.global_clock})
    )
    si = drain_inst.ins.sync_info
    waits = list(si.on_wait) if si and si.on_wait else []
    if len(waits) > 1:
        drain_inst.ins.sync_info = mybir.SyncInfo(
            on_wait=[waits[0]], on_update=list(si.on_update or [])
        )
        for w in waits[1:]:
            nop = self.nc.sync.nop(nofuse=True)
            nop.ins.sync_info = mybir.SyncInfo(on_wait=[w], on_update=[])
    self.nc.all_engine_barrier()
    popped = self.nc._tile_sem_poison_stack.pop()
    assert popped is self._sem_poison
    self.nc.clear_and_free_semaphores(list(self.sems.allocated().values()))
    self.nc.all_engine_barrier()


tile.TileContext._drain_and_barrier = _patched_drain_and_barrier


def _split_excess_waits(nc, limit=1):
    # Workaround: this compiler allows only one sem wait on several
    # instruction encodings; move extra waits onto same-engine nops.
    eng_map = {
        mybir.EngineType.PE: nc.tensor,
        mybir.EngineType.Activation: nc.scalar,
        mybir.EngineType.DVE: nc.vector,
        mybir.EngineType.Pool: nc.gpsimd,
        mybir.EngineType.SP: nc.sync,
    }
    for blk in nc.cur_f.blocks:
        orig = list(blk.instructions)
        out = []
        for ins in orig:
            si = ins.sync_info
            waits = list(si.on_wait) if si and si.on_wait else []
            eng = eng_map.get(ins.engine)
            if len(waits) > limit and eng is not None:
                extra, keep = waits[:-limit], waits[-limit:]
                for w in extra:
                    nop = eng.nop(nofuse=True).ins
                    tail = nc.cur_f.blocks[-1].instructions
                    assert tail[-1] is nop
                    tail.pop()
                    nop.sync_info = mybir.SyncInfo(on_wait=[w], on_update=[])
                    out.append(nop)
                ins.sync_info = mybir.SyncInfo(
                    on_wait=keep, on_update=list(si.on_update or [])
                )
            out.append(ins)
        blk.instructions[:] = out

bf16 = ml_dtypes.bfloat16
BF = bass.mybir.dt.bfloat16
F32 = bass.mybir.dt.float32
AF = mybir.ActivationFunctionType
ALU = mybir.AluOpType

B, S, E, H, D = 2, 2048, 2048, 16, 128
BS = B * S
NCORES = 8
HPC = H // NCORES  # heads per core
DC = HPC * D  # per-core head-dim width (256)
SCALE = 1.0 / float(np.sqrt(D))
NEG = -1.0e9

TRACE = False
LAST_RESULTS = None
_NC_CACHE = None


def _build():
    nc = bass.Bass()
    xT = nc.declare_dram_parameter("xT", (E, BS), BF, isOutput=False)
    wqT = nc.declare_dram_parameter("wqT", (E, DC), BF, isOutput=False)
    wkT = nc.declare_dram_parameter("wkT", (E, DC), BF, isOutput=False)
    wvT = nc.declare_dram_parameter("wvT", (E, DC), BF, isOutput=False)
    woT = nc.declare_dram_parameter("woT", (DC, E), BF, isOutput=False)
    bqd = nc.declare_dram_parameter("bq", (DC, 1), F32, isOutput=False)
    bkd = nc.declare_dram_parameter("bk", (DC, 1), F32, isOutput=False)
    mskd = nc.declare_dram_parameter("mask", (512, 512), BF, isOutput=False)
    onkd = nc.declare_dram_parameter("onesk", (128, 128), BF, isOutput=False)
    yd = nc.declare_dram_parameter("y", (BS, E), F32, isOutput=True)

    with ExitStack() as ctx:
        tc = ctx.enter_context(tile.TileContext(nc))
        wp = ctx.enter_context(tc.tile_pool(name="wp", bufs=1))
        bp = ctx.enter_context(tc.tile_pool(name="bp", bufs=1))
        pp = ctx.enter_context(tc.tile_pool(name="pp", bufs=17))
        dp = ctx.enter_context(tc.tile_pool(name="dp", bufs=2))
        yp = ctx.enter_context(tc.tile_pool(name="yp", bufs=3))
        ps = ctx.enter_context(tc.tile_pool(name="ps", bufs=1, space="PSUM"))

        wq_sb = wp.tile([128, 16, DC], BF)
        wk_sb = wp.tile([128, 16, DC], BF)
        wv_sb = wp.tile([128, 16, DC], BF)
        wo_sb = wp.tile([128, HPC, E], BF)
        bq_sb = wp.tile([128, HPC, 1], F32)
        bk_sb = wp.tile([128, HPC, 1], F32)
        msk_sb = wp.tile([128, 4, 512], BF)
        onk_sb = wp.tile([128, 128], BF)

        for b in range(B):
            s0 = b * S
            x_sb = bp.tile([128, 16, S], BF)
            for t in range(16):
                nc.sync.dma_start(x_sb[:, t, :], xT[t * 128 : (t + 1) * 128, s0 : s0 + S])
                if b == 0:
                    # interleave weight loads with batch-0 x so the first
                    # projection chain starts as soon as its t-slice lands
                    nc.sync.dma_start(wq_sb[:, t, :], wqT[t * 128 : (t + 1) * 128, :])
                    nc.sync.dma_start(wk_sb[:, t, :], wkT[t * 128 : (t + 1) * 128, :])
            if b == 0:
                for h in range(HPC):
                    nc.sync.dma_start(bq_sb[:, h, :], bqd[h * 128 : (h + 1) * 128, :])
                    nc.sync.dma_start(bk_sb[:, h, :], bkd[h * 128 : (h + 1) * 128, :])
                for t in range(16):
                    nc.sync.dma_start(wv_sb[:, t, :], wvT[t * 128 : (t + 1) * 128, :])
                for r in range(4):
                    nc.sync.dma_start(msk_sb[:, r, :], mskd[r * 128 : (r + 1) * 128, :])
                nc.sync.dma_start(onk_sb[:], onkd[:])
                for h in range(HPC):
                    nc.sync.dma_start(wo_sb[:, h, :], woT[h * 128 : (h + 1) * 128, :])
            qT_sb = bp.tile([128, HPC, S], BF)
            kT_sb = bp.tile([128, HPC, S], BF)
            v_sb = bp.tile([128, 16, DC], BF)
            ctxN_sb = bp.tile([128, HPC, S], BF)

            # --- q/k/v projections (contract E in 16 chunks of 128) ---
            for m in range(HPC):
                for j in range(4):
                    js = slice(j * 512, (j + 1) * 512)
                    q_ps = ps.tile([128, 512], F32, tag="pr", bufs=2)
                    for t in range(16):
                        nc.tensor.matmul(
                            q_ps[:],
                            wq_sb[:, t, m * 128 : (m + 1) * 128],
                            x_sb[:, t, js],
                            start=(t == 0),
                            stop=(t == 15),
                        )
                    nc.scalar.activation(
                        qT_sb[:, m, js], q_ps[:], AF.Identity, bias=bq_sb[:, m, :]
                    )
                    k_ps = ps.tile([128, 512], F32, tag="pr", bufs=2)
                    for t in range(16):
                        nc.tensor.matmul(
                            k_ps[:],
                            wk_sb[:, t, m * 128 : (m + 1) * 128],
                            x_sb[:, t, js],
                            start=(t == 0),
                            stop=(t == 15),
                        )
                    nc.scalar.activation(
                        kT_sb[:, m, js], k_ps[:], AF.Identity, bias=bk_sb[:, m, :]
                    )
            for si in range(16):
                v_ps = ps.tile([128, DC], F32, tag="pr", bufs=2)
                for t in range(16):
                    nc.tensor.matmul(
                        v_ps[:],
                        x_sb[:, t, si * 128 : (si + 1) * 128],
                        wv_sb[:, t, :],
                        start=(t == 0),
                        stop=(t == 15),
                    )
                nc.vector.tensor_copy(v_sb[:, si, :], v_ps[:])

            # --- causal attention, scores kept transposed [k, q] ---
            for h in range(HPC):
                hd = slice(h * 128, (h + 1) * 128)
                for qb in range(4):
                    qs = slice(qb * 512, (qb + 1) * 512)
                    kmax = 4 * qb + 4
                    pts = []
                    den_ps = ps.tile([128, 512], F32, tag="dn", bufs=1)
                    ctx_ps = ps.tile([128, 512], F32, tag="cx", bufs=2)
                    LAG = 2
                    # interleave den/ctx accumulation (lagging LAG tiles)
                    # between score matmuls so PE never waits on ACT exp
                    for kc in range(kmax + LAG):
                        if kc < kmax:
                            sc_ps = ps.tile([128, 512], F32, tag="sc", bufs=3)
                            diag = kc - 4 * qb
                            nc.tensor.matmul(
                                sc_ps[:],
                                kT_sb[:, h, kc * 128 : (kc + 1) * 128],
                                qT_sb[:, h, qs],
                                start=True,
                                stop=True,
                            )
                            p_t = pp.tile([128, 512], BF)
                            nc.scalar.activation(p_t[:], sc_ps[:], AF.Exp)
                            if diag >= 0:
                                nc.vector.tensor_tensor(
                                    p_t[:], p_t[:], msk_sb[:, diag, :], ALU.mult
                                )
                            pts.append(p_t)
                        j = kc - LAG
                        if j >= 0:
                            nc.tensor.matmul(
                                den_ps[:],
                                onk_sb[:],
                                pts[j][:],
                                start=(j == 0),
                                stop=(j == kmax - 1),
                            )
                            nc.tensor.matmul(
                                ctx_ps[:],
                                v_sb[:, j, hd],
                                pts[j][:],
                                start=(j == 0),
                                stop=(j == kmax - 1),
                            )
                    lnd_sb = dp.tile([128, 512], F32, tag="lnd", bufs=2)
                    nc.scalar.activation(lnd_sb[:], den_ps[:], AF.Ln)
                    recb_sb = dp.tile([128, 512], F32, tag="recb", bufs=2)
                    nc.scalar.activation(recb_sb[:], lnd_sb[:], AF.Exp, scale=-1.0)
                    nc.vector.tensor_tensor(
                        ctxN_sb[:, h, qs], ctx_ps[:], recb_sb[:], ALU.mult
                    )

            # --- output projection (contract per-core d=256 in 2 head chunks) ---
            for qc in range(16):
                for eb in range(4):
                    y_ps = ps.tile([128, 512], F32, tag="pr", bufs=2)
                    nc.tensor.matmul(
                        y_ps[:],
                        ctxN_sb[:, 0, qc * 128 : (qc + 1) * 128],
                        wo_sb[:, 0, eb * 512 : (eb + 1) * 512],
                        start=True,
                        stop=False,
                    )
                    nc.tensor.matmul(
                        y_ps[:],
                        ctxN_sb[:, 1, qc * 128 : (qc + 1) * 128],
                        wo_sb[:, 1, eb * 512 : (eb + 1) * 512],
                        start=False,
                        stop=True,
                    )
                    y_t = yp.tile([128, 512], F32)
                    if (qc * 4 + eb) % 2 == 0:
                        nc.vector.tensor_copy(y_t[:], y_ps[:])
                    else:
                        nc.scalar.copy(y_t[:], y_ps[:])
                    nc.sync.dma_start(
                        yd[s0 + qc * 128 : s0 + (qc + 1) * 128, eb * 512 : (eb + 1) * 512],
                        y_t[:],
                    )
    _split_excess_waits(nc)
    return nc


def _mask_np():
    m = np.zeros((4, 128, 512), np.float32)
    kk = np.arange(128)[:, None]
    qq = np.arange(512)[None, :]
    for r in range(4):
        m[r] = np.where(kk + 128 * r > qq, 0.0, 1.0)
    return m.reshape(512, 512).astype(bf16)


def kernel(**inputs):
    global LAST_RESULTS, _NC_CACHE
    x = np.asarray(inputs["x"], np.float32)
    Wq = np.asarray(inputs["Wq"], np.float32)
    bq = np.asarray(inputs["bq"], np.float32)
    Wk = np.asarray(inputs["Wk"], np.float32)
    bk = np.asarray(inputs["bk"], np.float32)
    Wv = np.asarray(inputs["Wv"], np.float32)
    bv = np.asarray(inputs["bv"], np.float32)
    Wo = np.asarray(inputs["Wo"], np.float32)
    bo = np.asarray(inputs["bo"], np.float32)

    xT = np.ascontiguousarray(x.reshape(BS, E).T).astype(bf16)
    mask = _mask_np()
    onesk = np.ones((128, 128), bf16)

    in_maps = []
    for c in range(NCORES):
        dsl = slice(c * DC, (c + 1) * DC)
        in_maps.append(
            {
                "xT": xT,
                "wqT": np.ascontiguousarray(Wq[dsl].T * SCALE).astype(bf16),
                "wkT": np.ascontiguousarray(Wk[dsl].T).astype(bf16),
                "wvT": np.ascontiguousarray(Wv[dsl].T).astype(bf16),
                "woT": np.ascontiguousarray(Wo[:, dsl].T).astype(bf16),
                "bq": np.ascontiguousarray((bq[dsl] * SCALE).reshape(DC, 1)),
                "bk": np.ascontiguousarray(bk[dsl].reshape(DC, 1)),
                "mask": mask,
                "onesk": onesk,
            }
        )

    if _NC_CACHE is None:
        _NC_CACHE = _build()
    res = run_bass_kernel_spmd(_NC_CACHE, in_maps, core_ids=list(range(NCORES)), trace=TRACE)
    LAST_RESULTS = res

    acc = None
    for r in res.results:
        yc = np.asarray(r["y"], np.float32)
        acc = yc if acc is None else acc + yc
    bo_eff = bo + bv @ Wo.T
    acc += bo_eff[None, :]
    return acc.reshape(B, S, E).astype(np.float32)



# revision 4
# speedup vs baseline: 1.0039x; 1.0039x over previous
import sys

sys.path.insert(0, "/opt/trn_rl_repo")

from contextlib import ExitStack

import ml_dtypes
import numpy as np

from concourse import bass, mybir, tile
from concourse.bass_utils import run_bass_kernel_spmd
from concourse.vector_clock import ScopedClock


def _patched_drain_and_barrier(self, tick_clock, wait_clock):
    # Workaround: this compiler rejects a drain carrying >1 sem wait
    # ([NCC_INLA001]); split extra waits onto single-wait nops.
    drain_inst = self.nc.sync.drain()
    wait_clock.add_sem_waits(
        drain_inst.ins, ScopedClock({None: tick_clock.global_clock})
    )
    si = drain_inst.ins.sync_info
    waits = list(si.on_wait) if si and si.on_wait else []
    if len(waits) > 1:
        drain_inst.ins.sync_info = mybir.SyncInfo(
            on_wait=[waits[0]], on_update=list(si.on_update or [])
        )
        for w in waits[1:]:
            nop = self.nc.sync.nop(nofuse=True)
            nop.ins.sync_info = mybir.SyncInfo(on_wait=[w], on_update=[])
    self.nc.all_engine_barrier()
    popped = self.nc._tile_sem_poison_stack.pop()
    assert popped is self._sem_poison
    self.nc.clear_and_free_semaphores(list(self.sems.allocated().values()))
    self.nc.all_engine_barrier()


tile.TileContext._drain_and_barrier = _patched_drain_and_barrier


def _split_excess_waits(nc, limit=1):
    # Workaround: this compiler allows only one sem wait on several
    # instruction encodings; move extra waits onto same-engine nops.
    eng_map = {
        mybir.EngineType.PE: nc.tensor,
        mybir.EngineType.Activation: nc.scalar,
        mybir.EngineType.DVE: nc.vector,
        mybir.EngineType.Pool: nc.gpsimd,
        mybir.EngineType.SP: nc.sync,
    }
    for blk in nc.cur_f.blocks:
        orig = list(blk.instructions)
        out = []
        for ins in orig:
            si = ins.sync_info
            waits = list(si.on_wait) if si and si.on_wait else []
            eng = eng_map.get(ins.engine)
            if len(waits) > limit and eng is not None:
                extra, keep = waits[:-limit], waits[-limit:]
                for w in extra:
                    nop = eng.nop(nofuse=True).ins
                    tail = nc.cur_f.blocks[-1].instructions
                    assert tail[-1] is nop
                    tail.pop()
                    nop.sync_info = mybir.SyncInfo(on_wait=[w], on_update=[])
                    out.append(nop)
                ins.sync_info = mybir.SyncInfo(
                    on_wait=keep, on_update=list(si.on_update or [])
                )
            out.append(ins)
        blk.instructions[:] = out

bf16 = ml_dtypes.bfloat16
BF = bass.mybir.dt.bfloat16
F32 = bass.mybir.dt.float32
AF = mybir.ActivationFunctionType
ALU = mybir.AluOpType

B, S, E, H, D = 2, 2048, 2048, 16, 128
BS = B * S
NCORES = 8
HPC = H // NCORES  # heads per core
DC = HPC * D  # per-core head-dim width (256)
SCALE = 1.0 / float(np.sqrt(D))

TRACE = False
LAST_RESULTS = None
_NC_CACHE = None


def _build():
    nc = bass.Bass()
    xT = nc.declare_dram_parameter("xT", (E, BS), BF, isOutput=False)
    wqT = nc.declare_dram_parameter("wqT", (E, DC), BF, isOutput=False)
    wkT = nc.declare_dram_parameter("wkT", (E, DC), BF, isOutput=False)
    wvT = nc.declare_dram_parameter("wvT", (E, DC), BF, isOutput=False)
    woT = nc.declare_dram_parameter("woT", (DC, E), BF, isOutput=False)
    bqd = nc.declare_dram_parameter("bq", (DC, 1), F32, isOutput=False)
    identd = nc.declare_dram_parameter("ident", (128, 128), BF, isOutput=False)
    trid = nc.declare_dram_parameter("trineg", (128, 128), BF, isOutput=False)
    onkd = nc.declare_dram_parameter("onesk", (128, 128), BF, isOutput=False)
    yd = nc.declare_dram_parameter("y", (BS, E), BF, isOutput=True)

    with ExitStack() as ctx:
        tc = ctx.enter_context(tile.TileContext(nc))
        wp = ctx.enter_context(tc.tile_pool(name="wp", bufs=1))
        bp = ctx.enter_context(tc.tile_pool(name="bp", bufs=1))
        pp = ctx.enter_context(tc.tile_pool(name="pp", bufs=17))
        dp = ctx.enter_context(tc.tile_pool(name="dp", bufs=2))
        yp = ctx.enter_context(tc.tile_pool(name="yp", bufs=4))
        ps = ctx.enter_context(tc.tile_pool(name="ps", bufs=1, space="PSUM"))

        wq_sb = wp.tile([128, 16, DC], BF)
        wk_sb = wp.tile([128, 16, DC], BF)
        wv_sb = wp.tile([128, 16, DC], BF)
        wo_sb = wp.tile([128, HPC, E], BF)
        bq_sb = wp.tile([128, HPC, 1], F32)
        id_sb = wp.tile([128, 128], BF)
        tri_sb = wp.tile([128, 128], BF)
        onk_sb = wp.tile([128, 128], BF)

        for b in range(B):
            s0 = b * S
            x_sb = bp.tile([128, 16, S], BF, tag="x")
            # j-major x DMA so the first projection chain starts after ~2MB
            for j in range(4):
                js = slice(j * 512, (j + 1) * 512)
                for t in range(16):
                    nc.sync.dma_start(
                        x_sb[:, t, js],
                        xT[t * 128 : (t + 1) * 128, s0 + j * 512 : s0 + (j + 1) * 512],
                    )
                    if b == 0 and j == 0:
                        nc.sync.dma_start(wq_sb[:, t, :], wqT[t * 128 : (t + 1) * 128, :])
                        nc.sync.dma_start(wk_sb[:, t, :], wkT[t * 128 : (t + 1) * 128, :])
                    if b == 0 and j == 1:
                        nc.sync.dma_start(wv_sb[:, t, :], wvT[t * 128 : (t + 1) * 128, :])
                if b == 0 and j == 2:
                    for h in range(HPC):
                        nc.sync.dma_start(bq_sb[:, h, :], bqd[h * 128 : (h + 1) * 128, :])
                    nc.sync.dma_start(id_sb[:], identd[:])
                    nc.sync.dma_start(tri_sb[:], trid[:])
                    nc.sync.dma_start(onk_sb[:], onkd[:])
                if b == 0 and j == 3:
                    for h in range(HPC):
                        nc.sync.dma_start(wo_sb[:, h, :], woT[h * 128 : (h + 1) * 128, :])
            qT_sb = bp.tile([128, HPC, S], BF, tag="qT")
            kT_sb = bp.tile([128, HPC, S], BF, tag="kT")
            v_sb = bp.tile([128, 16, DC], BF, tag="v")
            ctxN_sb = bp.tile([128, HPC, S], BF, tag="cN", bufs=2)

            # --- q/k/v projections, j-major to follow the DMA order ---
            # k bias is skipped entirely: softmax is invariant to the
            # per-query constant q·bk it would add to every score row.
            for j in range(4):
                js = slice(j * 512, (j + 1) * 512)
                for m in range(HPC):
                    q_ps = ps.tile([128, 512], F32, tag="pr", bufs=2)
                    for t in range(16):
                        nc.tensor.matmul(
                            q_ps[:],
                            wq_sb[:, t, m * 128 : (m + 1) * 128],
                            x_sb[:, t, js],
                            start=(t == 0),
                            stop=(t == 15),
                        )
                    nc.scalar.activation(
                        qT_sb[:, m, js], q_ps[:], AF.Identity, bias=bq_sb[:, m, :]
                    )
                    k_ps = ps.tile([128, 512], F32, tag="pr", bufs=2)
                    for t in range(16):
                        nc.tensor.matmul(
                            k_ps[:],
                            wk_sb[:, t, m * 128 : (m + 1) * 128],
                            x_sb[:, t, js],
                            start=(t == 0),
                            stop=(t == 15),
                        )
                    nc.vector.tensor_copy(kT_sb[:, m, js], k_ps[:])
                for si in range(4 * j, 4 * j + 4):
                    v_ps = ps.tile([128, DC], F32, tag="pr", bufs=2)
                    for t in range(16):
                        nc.tensor.matmul(
                            v_ps[:],
                            x_sb[:, t, si * 128 : (si + 1) * 128],
                            wv_sb[:, t, :],
                            start=(t == 0),
                            stop=(t == 15),
                        )
                    nc.vector.tensor_copy(v_sb[:, si, :], v_ps[:])

            # --- causal attention, scores kept transposed [k, q] ---
            # Valid-column trim: chunk kc only contributes to queries
            # q >= kc*128, so all score/exp/den/ctx work runs on the
            # [off:512] column slice. Masking of the within-block upper
            # triangle is done by accumulating -64 into the score psum
            # (identity-stationary matmul), so exp kills it for free.
            for h in range(HPC):
                hd = slice(h * 128, (h + 1) * 128)
                for qb in range(4):
                    q0 = qb * 512
                    kmax = 4 * qb + 4
                    pts = []
                    den_ps = ps.tile([128, 512], F32, tag="dn", bufs=1)
                    ctx_ps = ps.tile([128, 512], F32, tag="cx", bufs=2)
                    LAG = 2
                    # interleave den/ctx accumulation (lagging LAG tiles)
                    # between score matmuls so PE never waits on ACT exp
                    for kc in range(kmax + LAG):
                        if kc < kmax:
                            diag = kc - 4 * qb
                            off = max(0, 128 * diag)
                            sc_ps = ps.tile([128, 512], F32, tag="sc", bufs=3)
                            nc.tensor.matmul(
                                sc_ps[:, off:512],
                                kT_sb[:, h, kc * 128 : (kc + 1) * 128],
                                qT_sb[:, h, q0 + off : q0 + 512],
                                start=True,
                                stop=(diag < 0),
                            )
                            if diag >= 0:
                                nc.tensor.matmul(
                                    sc_ps[:, off : off + 128],
                                    id_sb[:],
                                    tri_sb[:],
                                    start=False,
                                    stop=True,
                                    skip_group_check=True,
                                )
                            p_t = pp.tile([128, 512], BF)
                            nc.scalar.activation(
                                p_t[:, off:512], sc_ps[:, off:512], AF.Exp
                            )
                            pts.append((p_t, off))
                        j = kc - LAG
                        if j >= 0:
                            pj, oj = pts[j]
                            nc.tensor.matmul(
                                den_ps[:, oj:512],
                                onk_sb[:],
                                pj[:, oj:512],
                                start=(j == 0),
                                stop=(j == kmax - 1),
                                skip_group_check=True,
                            )
                            nc.tensor.matmul(
                                ctx_ps[:, oj:512],
                                v_sb[:, j, hd],
                                pj[:, oj:512],
                                start=(j == 0),
                                stop=(j == kmax - 1),
                                skip_group_check=True,
                            )
                    recb_sb = dp.tile([128, 512], F32, tag="recb", bufs=2)
                    nc.vector.reciprocal(recb_sb[:], den_ps[:])
                    nc.vector.tensor_tensor(
                        ctxN_sb[:, h, q0 : q0 + 512], ctx_ps[:], recb_sb[:], ALU.mult
                    )

            # --- output projection (contract per-core d=256 in 2 head chunks) ---
            for qc in range(16):
                for eb in range(4):
                    y_ps = ps.tile([128, 512], F32, tag="pr", bufs=2)
                    nc.tensor.matmul(
                        y_ps[:],
                        ctxN_sb[:, 0, qc * 128 : (qc + 1) * 128],
                        wo_sb[:, 0, eb * 512 : (eb + 1) * 512],
                        start=True,
                        stop=False,
                    )
                    nc.tensor.matmul(
                        y_ps[:],
                        ctxN_sb[:, 1, qc * 128 : (qc + 1) * 128],
                        wo_sb[:, 1, eb * 512 : (eb + 1) * 512],
                        start=False,
                        stop=True,
                    )
                    y_t = yp.tile([128, 512], BF)
                    if (qc * 4 + eb) % 2 == 0:
                        nc.vector.tensor_copy(y_t[:], y_ps[:])
                    else:
                        nc.scalar.copy(y_t[:], y_ps[:])
                    nc.sync.dma_start(
                        yd[s0 + qc * 128 : s0 + (qc + 1) * 128, eb * 512 : (eb + 1) * 512],
                        y_t[:],
                    )
    _split_excess_waits(nc)
    return nc


def kernel(**inputs):
    global LAST_RESULTS, _NC_CACHE
    x = np.asarray(inputs["x"], np.float32)
    Wq = np.asarray(inputs["Wq"], np.float32)
    bq = np.asarray(inputs["bq"], np.float32)
    Wk = np.asarray(inputs["Wk"], np.float32)
    bk = np.asarray(inputs["bk"], np.float32)
    Wv = np.asarray(inputs["Wv"], np.float32)
    bv = np.asarray(inputs["bv"], np.float32)
    Wo = np.asarray(inputs["Wo"], np.float32)
    bo = np.asarray(inputs["bo"], np.float32)

    xT = np.ascontiguousarray(x.reshape(BS, E).T).astype(bf16)
    onesk = np.ones((128, 128), bf16)
    ident = np.eye(128, dtype=bf16)
    kk = np.arange(128)[:, None]
    qq = np.arange(128)[None, :]
    trineg = np.where(kk > qq, -64.0, 0.0).astype(bf16)

    in_maps = []
    for c in range(NCORES):
        dsl = slice(c * DC, (c + 1) * DC)
        in_maps.append(
            {
                "xT": xT,
                "wqT": np.ascontiguousarray(Wq[dsl].T * SCALE).astype(bf16),
                "wkT": np.ascontiguousarray(Wk[dsl].T).astype(bf16),
                "wvT": np.ascontiguousarray(Wv[dsl].T).astype(bf16),
                "woT": np.ascontiguousarray(Wo[:, dsl].T).astype(bf16),
                "bq": np.ascontiguousarray((bq[dsl] * SCALE).reshape(DC, 1)),
                "ident": ident,
                "trineg": trineg,
                "onesk": onesk,
            }
        )

    if _NC_CACHE is None:
        _NC_CACHE = _build()
    res = run_bass_kernel_spmd(_NC_CACHE, in_maps, core_ids=list(range(NCORES)), trace=TRACE)
    LAST_RESULTS = res

    acc = None
    for r in res.results:
        yc = np.asarray(r["y"], np.float32)
        acc = yc if acc is None else acc + yc
    bo_eff = bo + bv @ Wo.T
    acc += bo_eff[None, :]
    return acc.reshape(B, S, E).astype(np.float32)


# revision 5
# speedup vs baseline: 1.1351x; 1.1307x over previous
import sys

sys.path.insert(0, "/opt/trn_rl_repo")

from contextlib import ExitStack

import ml_dtypes
import numpy as np

from concourse import bass, mybir, tile
from concourse.bass_utils import run_bass_kernel_spmd
from concourse.vector_clock import ScopedClock


def _patched_drain_and_barrier(self, tick_clock, wait_clock):
    # Workaround: this compiler rejects a drain carrying >1 sem wait
    # ([NCC_INLA001]); split extra waits onto single-wait nops.
    drain_inst = self.nc.sync.drain()
    wait_clock.add_sem_waits(
        drain_inst.ins, ScopedClock({None: tick_clock.global_clock})
    )
    si = drain_inst.ins.sync_info
    waits = list(si.on_wait) if si and si.on_wait else []
    if len(waits) > 1:
        drain_inst.ins.sync_info = mybir.SyncInfo(
            on_wait=[waits[0]], on_update=list(si.on_update or [])
        )
        for w in waits[1:]:
            nop = self.nc.sync.nop(nofuse=True)
            nop.ins.sync_info = mybir.SyncInfo(on_wait=[w], on_update=[])
    self.nc.all_engine_barrier()
    popped = self.nc._tile_sem_poison_stack.pop()
    assert popped is self._sem_poison
    self.nc.clear_and_free_semaphores(list(self.sems.allocated().values()))
    self.nc.all_engine_barrier()


tile.TileContext._drain_and_barrier = _patched_drain_and_barrier


def _split_excess_waits(nc, limit=1):
    # Workaround: this compiler allows only one sem wait on several
    # instruction encodings; move extra waits onto same-engine nops.
    eng_map = {
        mybir.EngineType.PE: nc.tensor,
        mybir.EngineType.Activation: nc.scalar,
        mybir.EngineType.DVE: nc.vector,
        mybir.EngineType.Pool: nc.gpsimd,
        mybir.EngineType.SP: nc.sync,
    }
    for blk in nc.cur_f.blocks:
        orig = list(blk.instructions)
        out = []
        for ins in orig:
            si = ins.sync_info
            waits = list(si.on_wait) if si and si.on_wait else []
            eng = eng_map.get(ins.engine)
            if len(waits) > limit and eng is not None:
                extra, keep = waits[:-limit], waits[-limit:]
                for w in extra:
                    nop = eng.nop(nofuse=True).ins
                    tail = nc.cur_f.blocks[-1].instructions
                    assert tail[-1] is nop
                    tail.pop()
                    nop.sync_info = mybir.SyncInfo(on_wait=[w], on_update=[])
                    out.append(nop)
                ins.sync_info = mybir.SyncInfo(
                    on_wait=keep, on_update=list(si.on_update or [])
                )
            out.append(ins)
        blk.instructions[:] = out

bf16 = ml_dtypes.bfloat16
BF = bass.mybir.dt.bfloat16
F32 = bass.mybir.dt.float32
AF = mybir.ActivationFunctionType
ALU = mybir.AluOpType

B, S, E, H, D = 2, 2048, 2048, 16, 128
BS = B * S
NCORES = 8
HPC = H // NCORES  # heads per core
DC = HPC * D  # per-core head-dim width (256)
SCALE = 1.0 / float(np.sqrt(D))

TRACE = False
LAST_RESULTS = None
_NC_CACHE = None


def _build():
    nc = bass.Bass()
    xT = nc.declare_dram_parameter("xT", (E, BS), BF, isOutput=False)
    # weights pre-packed on host to [128, ...] so each is ONE contiguous DMA
    wq3 = nc.declare_dram_parameter("wq3", (128, 16 * DC), BF, isOutput=False)
    wk3 = nc.declare_dram_parameter("wk3", (128, 16 * DC), BF, isOutput=False)
    wv3 = nc.declare_dram_parameter("wv3", (128, 16 * DC), BF, isOutput=False)
    wo3 = nc.declare_dram_parameter("wo3", (128, HPC * E), BF, isOutput=False)
    bqd = nc.declare_dram_parameter("bq", (128, HPC), F32, isOutput=False)
    mscd = nc.declare_dram_parameter("misc", (128, 2 * 128), BF, isOutput=False)
    yd = nc.declare_dram_parameter("y", (BS, E), BF, isOutput=True)

    with ExitStack() as ctx:
        tc = ctx.enter_context(tile.TileContext(nc))
        wp = ctx.enter_context(tc.tile_pool(name="wp", bufs=1))
        bp = ctx.enter_context(tc.tile_pool(name="bp", bufs=1))
        pp = ctx.enter_context(tc.tile_pool(name="pp", bufs=8))
        dp = ctx.enter_context(tc.tile_pool(name="dp", bufs=2))
        yp = ctx.enter_context(tc.tile_pool(name="yp", bufs=6))
        ps = ctx.enter_context(tc.tile_pool(name="ps", bufs=1, space="PSUM"))

        wq_sb = wp.tile([128, 16, DC], BF)
        wk_sb = wp.tile([128, 16, DC], BF)
        wv_sb = wp.tile([128, 16, DC], BF)
        wo_sb = wp.tile([128, HPC, E], BF)
        bq_sb = wp.tile([128, HPC], F32)
        msc_sb = wp.tile([128, 2, 128], BF)
        msk_sb = msc_sb[:, 0, :]  # 0/1 lower-tri (k<=q) [k,q]
        onk_sb = msc_sb[:, 1, :]  # ones

        for b in range(B):
            s0 = b * S
            x_sb = bp.tile([128, 16, S], BF, tag="x")
            if b == 0:
                nc.sync.dma_start(wq_sb[:], wq3[:].rearrange("p (t d) -> p t d", t=16))
                nc.sync.dma_start(wk_sb[:], wk3[:].rearrange("p (t d) -> p t d", t=16))
            for t in range(16):
                nc.sync.dma_start(x_sb[:, t, :], xT[t * 128 : (t + 1) * 128, s0 : s0 + S])
                if b == 0 and t == 0:
                    nc.sync.dma_start(
                        wv_sb[:], wv3[:].rearrange("p (t d) -> p t d", t=16)
                    )
                if b == 0 and t == 1:
                    nc.sync.dma_start(bq_sb[:], bqd[:])
                    nc.sync.dma_start(
                        msc_sb[:], mscd[:].rearrange("p (a k) -> p a k", a=2)
                    )
                if b == 0 and t == 2:
                    nc.sync.dma_start(
                        wo_sb[:], wo3[:].rearrange("p (h e) -> p h e", h=HPC)
                    )
            qT_sb = bp.tile([128, HPC, S], BF, tag="qT")
            kT_sb = bp.tile([128, HPC, S], BF, tag="kT")
            v_sb = bp.tile([128, 16, DC], BF, tag="v")
            ctxN_sb = bp.tile([128, HPC, S], BF, tag="cN", bufs=2)

            # --- q/k/v projections (contract E in 16 chunks of 128) ---
            # k bias is skipped entirely: softmax is invariant to the
            # per-query constant q·bk it would add to every score row.
            for j in range(4):
                js = slice(j * 512, (j + 1) * 512)
                for m in range(HPC):
                    q_ps = ps.tile([128, 512], F32, tag="pr", bufs=2)
                    for t in range(16):
                        nc.tensor.matmul(
                            q_ps[:],
                            wq_sb[:, t, m * 128 : (m + 1) * 128],
                            x_sb[:, t, js],
                            start=(t == 0),
                            stop=(t == 15),
                        )
                    nc.scalar.activation(
                        qT_sb[:, m, js], q_ps[:], AF.Identity,
                        bias=bq_sb[:, m : m + 1],
                    )
                    k_ps = ps.tile([128, 512], F32, tag="pr", bufs=2)
                    for t in range(16):
                        nc.tensor.matmul(
                            k_ps[:],
                            wk_sb[:, t, m * 128 : (m + 1) * 128],
                            x_sb[:, t, js],
                            start=(t == 0),
                            stop=(t == 15),
                        )
                    nc.vector.tensor_copy(kT_sb[:, m, js], k_ps[:])
                for si in range(4 * j, 4 * j + 4):
                    v_ps = ps.tile([128, DC], F32, tag="pr", bufs=2)
                    for t in range(16):
                        nc.tensor.matmul(
                            v_ps[:],
                            x_sb[:, t, si * 128 : (si + 1) * 128],
                            wv_sb[:, t, :],
                            start=(t == 0),
                            stop=(t == 15),
                        )
                    nc.vector.tensor_copy(v_sb[:, si, :], v_ps[:])

            # --- causal attention, scores kept transposed [k, q] ---
            # Valid-column trim: chunk kc only contributes to queries
            # q >= kc*128, so all score/exp/den/ctx work runs on the
            # [off:512] column slice. qb outer / h inner so the two heads'
            # pipelines hide each other's boundary bubbles.
            for qb in range(4):
                q0 = qb * 512
                kmax = 4 * qb + 4
                for h in range(HPC):
                    hd = slice(h * 128, (h + 1) * 128)
                    pts = []
                    den_ps = ps.tile([128, 512], F32, tag="pr", bufs=2)
                    ctx_ps = ps.tile([128, 512], F32, tag="cx", bufs=3)
                    LAG = 2
                    # interleave den/ctx accumulation (lagging LAG tiles)
                    # between score matmuls so PE never waits on ACT exp
                    for kc in range(kmax + LAG):
                        if kc < kmax:
                            diag = kc - 4 * qb
                            off = max(0, 128 * diag)
                            sc_ps = ps.tile([128, 512], F32, tag="sc", bufs=3)
                            nc.tensor.matmul(
                                sc_ps[:, off:512],
                                kT_sb[:, h, kc * 128 : (kc + 1) * 128],
                                qT_sb[:, h, q0 + off : q0 + 512],
                                start=True,
                                stop=True,
                            )
                            p_t = pp.tile([128, 512], BF)
                            nc.scalar.activation(
                                p_t[:, off:512], sc_ps[:, off:512], AF.Exp
                            )
                            if diag >= 0:
                                nc.vector.tensor_tensor(
                                    p_t[:, off : off + 128],
                                    p_t[:, off : off + 128],
                                    msk_sb,
                                    ALU.mult,
                                )
                            pts.append((p_t, off))
                        j = kc - LAG
                        if j >= 0:
                            pj, oj = pts[j]
                            nc.tensor.matmul(
                                den_ps[:, oj:512],
                                onk_sb,
                                pj[:, oj:512],
                                start=(j == 0),
                                stop=(j == kmax - 1),
                                skip_group_check=True,
                            )
                            nc.tensor.matmul(
                                ctx_ps[:, oj:512],
                                v_sb[:, j, hd],
                                pj[:, oj:512],
                                start=(j == 0),
                                stop=(j == kmax - 1),
                                skip_group_check=True,
                            )
                    recb_sb = dp.tile([128, 512], F32, tag="recb", bufs=2)
                    nc.vector.reciprocal(recb_sb[:], den_ps[:])
                    nc.vector.tensor_tensor(
                        ctxN_sb[:, h, q0 : q0 + 512], ctx_ps[:], recb_sb[:], ALU.mult
                    )

            # --- output projection (contract per-core d=256 in 2 head chunks) ---
            for qc in range(16):
                for ep in range(2):
                    y_t = yp.tile([128, 1024], BF)
                    for ei in range(2):
                        eb = ep * 2 + ei
                        y_ps = ps.tile([128, 512], F32, tag="pr", bufs=2)
                        nc.tensor.matmul(
                            y_ps[:],
                            ctxN_sb[:, 0, qc * 128 : (qc + 1) * 128],
                            wo_sb[:, 0, eb * 512 : (eb + 1) * 512],
                            start=True,
                            stop=False,
                        )
                        nc.tensor.matmul(
                            y_ps[:],
                            ctxN_sb[:, 1, qc * 128 : (qc + 1) * 128],
                            wo_sb[:, 1, eb * 512 : (eb + 1) * 512],
                            start=False,
                            stop=True,
                        )
                        if eb % 2 == 0:
                            nc.vector.tensor_copy(
                                y_t[:, ei * 512 : (ei + 1) * 512], y_ps[:]
                            )
                        else:
                            nc.scalar.copy(y_t[:, ei * 512 : (ei + 1) * 512], y_ps[:])
                    nc.sync.dma_start(
                        yd[
                            s0 + qc * 128 : s0 + (qc + 1) * 128,
                            ep * 1024 : (ep + 1) * 1024,
                        ],
                        y_t[:],
                    )
    _split_excess_waits(nc)
    return nc


def _pack_w(w):
    # [E, DC] -> [128, 16*DC] so row p holds w[t*128+p, :] for t=0..15
    return np.ascontiguousarray(
        w.reshape(16, 128, DC).transpose(1, 0, 2).reshape(128, 16 * DC)
    )


def kernel(**inputs):
    global LAST_RESULTS, _NC_CACHE
    x = np.asarray(inputs["x"], np.float32)
    Wq = np.asarray(inputs["Wq"], np.float32)
    bq = np.asarray(inputs["bq"], np.float32)
    Wk = np.asarray(inputs["Wk"], np.float32)
    bk = np.asarray(inputs["bk"], np.float32)
    Wv = np.asarray(inputs["Wv"], np.float32)
    bv = np.asarray(inputs["bv"], np.float32)
    Wo = np.asarray(inputs["Wo"], np.float32)
    bo = np.asarray(inputs["bo"], np.float32)

    xT = np.ascontiguousarray(x.reshape(BS, E).T).astype(bf16)
    kk = np.arange(128)[:, None]
    qq = np.arange(128)[None, :]
    misc = np.concatenate(
        [
            np.where(kk <= qq, 1.0, 0.0),
            np.ones((128, 128)),
        ],
        axis=1,
    ).astype(bf16)

    in_maps = []
    for c in range(NCORES):
        dsl = slice(c * DC, (c + 1) * DC)
        in_maps.append(
            {
                "xT": xT,
                "wq3": _pack_w((Wq[dsl].T * SCALE).astype(bf16)),
                "wk3": _pack_w(Wk[dsl].T.astype(bf16)),
                "wv3": _pack_w(Wv[dsl].T.astype(bf16)),
                "wo3": np.ascontiguousarray(
                    Wo[:, dsl].T.astype(bf16)
                    .reshape(HPC, 128, E)
                    .transpose(1, 0, 2)
                    .reshape(128, HPC * E)
                ),
                "bq": np.ascontiguousarray(
                    (bq[dsl] * SCALE).astype(np.float32).reshape(HPC, 128).T
                ),
                "misc": misc,
            }
        )

    if _NC_CACHE is None:
        _NC_CACHE = _build()
    res = run_bass_kernel_spmd(_NC_CACHE, in_maps, core_ids=list(range(NCORES)), trace=TRACE)
    LAST_RESULTS = res

    acc = None
    for r in res.results:
        yc = np.asarray(r["y"], np.float32)
        acc = yc if acc is None else acc + yc
    bo_eff = bo + bv @ Wo.T
    acc += bo_eff[None, :]
    return acc.reshape(B, S, E).astype(np.float32)


# revision 10
# speedup vs baseline: 1.3398x; 1.1804x over previous
import sys

sys.path.insert(0, "/opt/trn_rl_repo")

from contextlib import ExitStack

import ml_dtypes
import numpy as np

from concourse import bass, mybir, tile
from concourse.bass_utils import run_bass_kernel_spmd
from concourse.vector_clock import ScopedClock


def _patched_drain_and_barrier(self, tick_clock, wait_clock):
    # Workaround: this compiler rejects a drain carrying >1 sem wait
    # ([NCC_INLA001]); split extra waits onto single-wait nops.
    drain_inst = self.nc.sync.drain()
    wait_clock.add_sem_waits(
        drain_inst.ins, ScopedClock({None: tick_clock.global_clock})
    )
    si = drain_inst.ins.sync_info
    waits = list(si.on_wait) if si and si.on_wait else []
    if len(waits) > 1:
        drain_inst.ins.sync_info = mybir.SyncInfo(
            on_wait=[waits[0]], on_update=list(si.on_update or [])
        )
        for w in waits[1:]:
            nop = self.nc.sync.nop(nofuse=True)
            nop.ins.sync_info = mybir.SyncInfo(on_wait=[w], on_update=[])
    self.nc.all_engine_barrier()
    popped = self.nc._tile_sem_poison_stack.pop()
    assert popped is self._sem_poison
    self.nc.clear_and_free_semaphores(list(self.sems.allocated().values()))
    self.nc.all_engine_barrier()


tile.TileContext._drain_and_barrier = _patched_drain_and_barrier


def _split_excess_waits(nc, limit=1):
    # Workaround: this compiler allows only one sem wait on several
    # instruction encodings; move extra waits onto same-engine nops.
    eng_map = {
        mybir.EngineType.PE: nc.tensor,
        mybir.EngineType.Activation: nc.scalar,
        mybir.EngineType.DVE: nc.vector,
        mybir.EngineType.Pool: nc.gpsimd,
        mybir.EngineType.SP: nc.sync,
    }
    for blk in nc.cur_f.blocks:
        orig = list(blk.instructions)
        out = []
        for ins in orig:
            si = ins.sync_info
            waits = list(si.on_wait) if si and si.on_wait else []
            eng = eng_map.get(ins.engine)
            if len(waits) > limit and eng is not None:
                extra, keep = waits[:-limit], waits[-limit:]
                for w in extra:
                    nop = eng.nop(nofuse=True).ins
                    tail = nc.cur_f.blocks[-1].instructions
                    assert tail[-1] is nop
                    tail.pop()
                    nop.sync_info = mybir.SyncInfo(on_wait=[w], on_update=[])
                    out.append(nop)
                ins.sync_info = mybir.SyncInfo(
                    on_wait=keep, on_update=list(si.on_update or [])
                )
            out.append(ins)
        blk.instructions[:] = out

bf16 = ml_dtypes.bfloat16
BF = bass.mybir.dt.bfloat16
F32 = bass.mybir.dt.float32
AF = mybir.ActivationFunctionType
ALU = mybir.AluOpType

B, S, E, H, D = 2, 2048, 2048, 16, 128
BS = B * S
NCORES = 8
HPC = H // NCORES  # heads per core
DC = HPC * D  # per-core head-dim width (256)
SCALE = 1.0 / float(np.sqrt(D))

TRACE = False
LAST_RESULTS = None
_NC_CACHE = None


def _build():
    nc = bass.Bass()
    xT = nc.declare_dram_parameter("xT", (E, BS), BF, isOutput=False)
    # weights pre-packed on host to [128, ...] so each is ONE contiguous DMA
    wq3 = nc.declare_dram_parameter("wq3", (128, 16 * DC), BF, isOutput=False)
    wk3 = nc.declare_dram_parameter("wk3", (128, 16 * DC), BF, isOutput=False)
    wv3 = nc.declare_dram_parameter("wv3", (128, 16 * DC), BF, isOutput=False)
    wo3 = nc.declare_dram_parameter("wo3", (128, HPC * E), BF, isOutput=False)
    bqd = nc.declare_dram_parameter("bq", (128, HPC), F32, isOutput=False)
    mscd = nc.declare_dram_parameter("misc", (128, 2 * 128), BF, isOutput=False)
    yd = nc.declare_dram_parameter("y", (BS, E), BF, isOutput=True)

    with ExitStack() as ctx:
        tc = ctx.enter_context(tile.TileContext(nc))
        wp = ctx.enter_context(tc.tile_pool(name="wp", bufs=1))
        bp = ctx.enter_context(tc.tile_pool(name="bp", bufs=1))
        pp = ctx.enter_context(tc.tile_pool(name="pp", bufs=8))
        dp = ctx.enter_context(tc.tile_pool(name="dp", bufs=2))
        yp = ctx.enter_context(tc.tile_pool(name="yp", bufs=6))
        ps = ctx.enter_context(tc.tile_pool(name="ps", bufs=1, space="PSUM"))

        wq_sb = wp.tile([128, 16, DC], BF)
        wk_sb = wp.tile([128, 16, DC], BF)
        wv_sb = wp.tile([128, 16, DC], BF)
        wo_sb = wp.tile([128, HPC, E], BF)
        bq_sb = wp.tile([128, HPC], F32)
        msc_sb = wp.tile([128, 2, 128], BF)
        msk_sb = msc_sb[:, 0, :]  # 0/1 lower-tri (k<=q) [k,q]
        onk_sb = msc_sb[:, 1, :]  # ones

        for b in range(B):
            s0 = b * S
            x_sb = bp.tile([128, 16, S], BF, tag="x")
            if b == 0:
                nc.sync.dma_start(wq_sb[:], wq3[:].rearrange("p (t d) -> p t d", t=16))
                nc.sync.dma_start(wk_sb[:], wk3[:].rearrange("p (t d) -> p t d", t=16))
                nc.sync.dma_start(wv_sb[:], wv3[:].rearrange("p (t d) -> p t d", t=16))
            for t in range(16):
                nc.sync.dma_start(x_sb[:, t, :], xT[t * 128 : (t + 1) * 128, s0 : s0 + S])
                if b == 0 and t == 1:
                    nc.sync.dma_start(bq_sb[:], bqd[:])
                    nc.sync.dma_start(
                        msc_sb[:], mscd[:].rearrange("p (a k) -> p a k", a=2)
                    )
                if b == 0 and t == 15:
                    nc.sync.dma_start(
                        wo_sb[:], wo3[:].rearrange("p (h e) -> p h e", h=HPC)
                    )
            qT_sb = bp.tile([128, HPC, S], BF, tag="qT")
            kT_sb = bp.tile([128, HPC, S], BF, tag="kT")
            v_sb = bp.tile([128, 16, DC], BF, tag="v")
            ctxN_sb = bp.tile([128, HPC, S], BF, tag="cN", bufs=2)

            # --- q/k/v projections (contract E in 16 chunks of 128) ---
            # Wave-major: all 8 chains of a j-block accumulate in parallel
            # across the 8 PSUM banks, inner loop over t, so PE advances
            # with the x DMA wave instead of stalling chain-by-chain.
            # k bias is skipped entirely: softmax is invariant to the
            # per-query constant q·bk it would add to every score row.
            for j in range(4):
                js = slice(j * 512, (j + 1) * 512)
                tags = [("pr", 2), ("pr", 2), ("sc", 3), ("sc", 3),
                        ("sc", 3), ("cx", 3), ("cx", 3), ("cx", 3)]
                qk_ps = []
                for m in range(HPC):
                    tg, bf = tags[2 * m], tags[2 * m + 1]
                    qm_ps = ps.tile([128, 512], F32, tag=tg[0], bufs=tg[1], name=f"q{m}")
                    km_ps = ps.tile([128, 512], F32, tag=bf[0], bufs=bf[1], name=f"k{m}")
                    qk_ps.append(qm_ps)
                    qk_ps.append(km_ps)
                v_pss = []
                for si in range(4):
                    vs_ps = ps.tile(
                        [128, DC], F32, tag=tags[4 + si][0], bufs=tags[4 + si][1],
                        name=f"v{si}",
                    )
                    v_pss.append(vs_ps)
                for t in range(16):
                    for m in range(HPC):
                        nc.tensor.matmul(
                            qk_ps[2 * m][:],
                            wq_sb[:, t, m * 128 : (m + 1) * 128],
                            x_sb[:, t, js],
                            start=(t == 0),
                            stop=(t == 15),
                        )
                        nc.tensor.matmul(
                            qk_ps[2 * m + 1][:],
                            wk_sb[:, t, m * 128 : (m + 1) * 128],
                            x_sb[:, t, js],
                            start=(t == 0),
                            stop=(t == 15),
                        )
                    for si in range(4):
                        nc.tensor.matmul(
                            v_pss[si][:],
                            x_sb[:, t, (4 * j + si) * 128 : (4 * j + si + 1) * 128],
                            wv_sb[:, t, :],
                            start=(t == 0),
                            stop=(t == 15),
                        )
                for m in range(HPC):
                    nc.scalar.activation(
                        qT_sb[:, m, js], qk_ps[2 * m][:], AF.Identity,
                        bias=bq_sb[:, m : m + 1],
                    )
                    nc.vector.tensor_copy(kT_sb[:, m, js], qk_ps[2 * m + 1][:])
                for si in range(4):
                    if si == 3:
                        nc.scalar.copy(v_sb[:, 4 * j + si, :], v_pss[si][:])
                    else:
                        eng = [nc.vector, nc.gpsimd, nc.gpsimd][si]
                        eng.tensor_copy(v_sb[:, 4 * j + si, :], v_pss[si][:])

            # --- causal attention, scores kept transposed [k, q] ---
            # Valid-column trim: chunk kc only contributes to queries
            # q >= kc*128, so all score/exp/den/ctx work runs on the
            # [off:512] column slice. qb outer / h inner so the two heads'
            # pipelines hide each other's boundary bubbles.
            for qb in range(4):
                q0 = qb * 512
                kmax = 4 * qb + 4
                for h in range(HPC):
                    hd = slice(h * 128, (h + 1) * 128)
                    pts = []
                    den_ps = ps.tile([128, 512], F32, tag="pr", bufs=2)
                    ctx_ps = ps.tile([128, 512], F32, tag="cx", bufs=3)
                    LAG = 2
                    # interleave den/ctx accumulation (lagging LAG tiles)
                    # between score matmuls so PE never waits on ACT exp
                    for kc in range(kmax + LAG):
                        if kc < kmax:
                            diag = kc - 4 * qb
                            off = max(0, 128 * diag)
                            sc_ps = ps.tile([128, 512], F32, tag="sc", bufs=3)
                            nc.tensor.matmul(
                                sc_ps[:, off:512],
                                kT_sb[:, h, kc * 128 : (kc + 1) * 128],
                                qT_sb[:, h, q0 + off : q0 + 512],
                                start=True,
                                stop=True,
                            )
                            p_t = pp.tile([128, 512], BF)
                            nc.scalar.activation(
                                p_t[:, off:512], sc_ps[:, off:512], AF.Exp
                            )
                            if diag >= 0:
                                nc.vector.tensor_tensor(
                                    p_t[:, off : off + 128],
                                    p_t[:, off : off + 128],
                                    msk_sb,
                                    ALU.mult,
                                )
                            pts.append((p_t, off))
                        j = kc - LAG
                        if j >= 0:
                            pj, oj = pts[j]
                            nc.tensor.matmul(
                                den_ps[:, oj:512],
                                onk_sb,
                                pj[:, oj:512],
                                start=(j == 0),
                                stop=(j == kmax - 1),
                                skip_group_check=True,
                            )
                            nc.tensor.matmul(
                                ctx_ps[:, oj:512],
                                v_sb[:, j, hd],
                                pj[:, oj:512],
                                start=(j == 0),
                                stop=(j == kmax - 1),
                                skip_group_check=True,
                            )
                    recb_sb = dp.tile([128, 512], F32, tag="recb", bufs=2)
                    nc.vector.reciprocal(recb_sb[:], den_ps[:])
                    nc.gpsimd.tensor_tensor(
                        ctxN_sb[:, h, q0 : q0 + 512], ctx_ps[:], recb_sb[:], ALU.mult
                    )

            # --- output projection (contract per-core d=256 in 2 head chunks) ---
            # y_ps rotates across all 3 psum tags; drains rotate across
            # DVE/ACT/Pool so copies never pace the PE.
            ytags = [("pr", 2), ("sc", 3), ("cx", 3)]
            yengs = [nc.vector, nc.scalar, nc.gpsimd]
            for qc in range(16):
                for ep in range(2):
                    y_t = yp.tile([128, 1024], BF)
                    for ei in range(2):
                        eb = ep * 2 + ei
                        n = qc * 4 + eb
                        tg = ytags[n % 3]
                        y_ps = ps.tile([128, 512], F32, tag=tg[0], bufs=tg[1])
                        nc.tensor.matmul(
                            y_ps[:],
                            ctxN_sb[:, 0, qc * 128 : (qc + 1) * 128],
                            wo_sb[:, 0, eb * 512 : (eb + 1) * 512],
                            start=True,
                            stop=False,
                        )
                        nc.tensor.matmul(
                            y_ps[:],
                            ctxN_sb[:, 1, qc * 128 : (qc + 1) * 128],
                            wo_sb[:, 1, eb * 512 : (eb + 1) * 512],
                            start=False,
                            stop=True,
                        )
                        eng = yengs[n % 3]
                        if eng is nc.scalar:
                            eng.copy(y_t[:, ei * 512 : (ei + 1) * 512], y_ps[:])
                        else:
                            eng.tensor_copy(y_t[:, ei * 512 : (ei + 1) * 512], y_ps[:])
                    nc.sync.dma_start(
                        yd[
                            s0 + qc * 128 : s0 + (qc + 1) * 128,
                            ep * 1024 : (ep + 1) * 1024,
                        ],
                        y_t[:],
                    )
    _split_excess_waits(nc)
    return nc


def _pack_w(w):
    # [E, DC] -> [128, 16*DC] so row p holds w[t*128+p, :] for t=0..15
    return np.ascontiguousarray(
        w.reshape(16, 128, DC).transpose(1, 0, 2).reshape(128, 16 * DC)
    )


def kernel(**inputs):
    global LAST_RESULTS, _NC_CACHE
    x = np.asarray(inputs["x"], np.float32)
    Wq = np.asarray(inputs["Wq"], np.float32)
    bq = np.asarray(inputs["bq"], np.float32)
    Wk = np.asarray(inputs["Wk"], np.float32)
    bk = np.asarray(inputs["bk"], np.float32)
    Wv = np.asarray(inputs["Wv"], np.float32)
    bv = np.asarray(inputs["bv"], np.float32)
    Wo = np.asarray(inputs["Wo"], np.float32)
    bo = np.asarray(inputs["bo"], np.float32)

    xT = np.ascontiguousarray(x.reshape(BS, E).T).astype(bf16)
    kk = np.arange(128)[:, None]
    qq = np.arange(128)[None, :]
    misc = np.concatenate(
        [
            np.where(kk <= qq, 1.0, 0.0),
            np.ones((128, 128)),
        ],
        axis=1,
    ).astype(bf16)

    in_maps = []
    for c in range(NCORES):
        dsl = slice(c * DC, (c + 1) * DC)
        in_maps.append(
            {
                "xT": xT,
                "wq3": _pack_w((Wq[dsl].T * SCALE).astype(bf16)),
                "wk3": _pack_w(Wk[dsl].T.astype(bf16)),
                "wv3": _pack_w(Wv[dsl].T.astype(bf16)),
                "wo3": np.ascontiguousarray(
                    Wo[:, dsl].T.astype(bf16)
                    .reshape(HPC, 128, E)
                    .transpose(1, 0, 2)
                    .reshape(128, HPC * E)
                ),
                "bq": np.ascontiguousarray(
                    (bq[dsl] * SCALE).astype(np.float32).reshape(HPC, 128).T
                ),
                "misc": misc,
            }
        )

    if _NC_CACHE is None:
        _NC_CACHE = _build()
    res = run_bass_kernel_spmd(_NC_CACHE, in_maps, core_ids=list(range(NCORES)), trace=TRACE)
    LAST_RESULTS = res

    acc = None
    for r in res.results:
        yc = np.asarray(r["y"], np.float32)
        acc = yc if acc is None else acc + yc
    bo_eff = bo + bv @ Wo.T
    acc += bo_eff[None, :]
    return acc.reshape(B, S, E).astype(np.float32)


# revision 11
# speedup vs baseline: 1.3430x; 1.0024x over previous
import sys

sys.path.insert(0, "/opt/trn_rl_repo")

from contextlib import ExitStack

import ml_dtypes
import numpy as np

from concourse import bass, mybir, tile
from concourse.bass_utils import run_bass_kernel_spmd
from concourse.vector_clock import ScopedClock


def _patched_drain_and_barrier(self, tick_clock, wait_clock):
    # Workaround: this compiler rejects a drain carrying >1 sem wait
    # ([NCC_INLA001]); split extra waits onto single-wait nops.
    drain_inst = self.nc.sync.drain()
    wait_clock.add_sem_waits(
        drain_inst.ins, ScopedClock({None: tick_clock.global_clock})
    )
    si = drain_inst.ins.sync_info
    waits = list(si.on_wait) if si and si.on_wait else []
    if len(waits) > 1:
        drain_inst.ins.sync_info = mybir.SyncInfo(
            on_wait=[waits[0]], on_update=list(si.on_update or [])
        )
        for w in waits[1:]:
            nop = self.nc.sync.nop(nofuse=True)
            nop.ins.sync_info = mybir.SyncInfo(on_wait=[w], on_update=[])
    self.nc.all_engine_barrier()
    popped = self.nc._tile_sem_poison_stack.pop()
    assert popped is self._sem_poison
    self.nc.clear_and_free_semaphores(list(self.sems.allocated().values()))
    self.nc.all_engine_barrier()


tile.TileContext._drain_and_barrier = _patched_drain_and_barrier


def _split_excess_waits(nc, limit=1):
    # Workaround: this compiler allows only one sem wait on several
    # instruction encodings; move extra waits onto same-engine nops.
    eng_map = {
        mybir.EngineType.PE: nc.tensor,
        mybir.EngineType.Activation: nc.scalar,
        mybir.EngineType.DVE: nc.vector,
        mybir.EngineType.Pool: nc.gpsimd,
        mybir.EngineType.SP: nc.sync,
    }
    for blk in nc.cur_f.blocks:
        orig = list(blk.instructions)
        out = []
        for ins in orig:
            si = ins.sync_info
            waits = list(si.on_wait) if si and si.on_wait else []
            eng = eng_map.get(ins.engine)
            if len(waits) > limit and eng is not None:
                extra, keep = waits[:-limit], waits[-limit:]
                for w in extra:
                    nop = eng.nop(nofuse=True).ins
                    tail = nc.cur_f.blocks[-1].instructions
                    assert tail[-1] is nop
                    tail.pop()
                    nop.sync_info = mybir.SyncInfo(on_wait=[w], on_update=[])
                    out.append(nop)
                ins.sync_info = mybir.SyncInfo(
                    on_wait=keep, on_update=list(si.on_update or [])
                )
            out.append(ins)
        blk.instructions[:] = out

bf16 = ml_dtypes.bfloat16
BF = bass.mybir.dt.bfloat16
F32 = bass.mybir.dt.float32
AF = mybir.ActivationFunctionType
ALU = mybir.AluOpType

B, S, E, H, D = 2, 2048, 2048, 16, 128
BS = B * S
NCORES = 8
HPC = H // NCORES  # heads per core
DC = HPC * D  # per-core head-dim width (256)
SCALE = 1.0 / float(np.sqrt(D))

TRACE = False
LAST_RESULTS = None
_NC_CACHE = None


def _build():
    nc = bass.Bass()
    xT = nc.declare_dram_parameter("xT", (E, BS), BF, isOutput=False)
    # weights pre-packed on host to [128, ...] so each is ONE contiguous DMA
    wq3 = nc.declare_dram_parameter("wq3", (128, 16 * DC), BF, isOutput=False)
    wk3 = nc.declare_dram_parameter("wk3", (128, 16 * DC), BF, isOutput=False)
    wv3 = nc.declare_dram_parameter("wv3", (128, 16 * DC), BF, isOutput=False)
    wo3 = nc.declare_dram_parameter("wo3", (128, HPC * E), BF, isOutput=False)
    bqd = nc.declare_dram_parameter("bq", (128, HPC), F32, isOutput=False)
    mscd = nc.declare_dram_parameter("misc", (128, 2 * 128), BF, isOutput=False)
    yd = nc.declare_dram_parameter("y", (BS, E), BF, isOutput=True)

    with ExitStack() as ctx:
        tc = ctx.enter_context(tile.TileContext(nc))
        wp = ctx.enter_context(tc.tile_pool(name="wp", bufs=1))
        bp = ctx.enter_context(tc.tile_pool(name="bp", bufs=1))
        pp = ctx.enter_context(tc.tile_pool(name="pp", bufs=8))
        dp = ctx.enter_context(tc.tile_pool(name="dp", bufs=2))
        yp = ctx.enter_context(tc.tile_pool(name="yp", bufs=6))
        ps = ctx.enter_context(tc.tile_pool(name="ps", bufs=1, space="PSUM"))

        wq_sb = wp.tile([128, 16, DC], BF)
        wk_sb = wp.tile([128, 16, DC], BF)
        wv_sb = wp.tile([128, 16, DC], BF)
        wo_sb = wp.tile([128, HPC, E], BF)
        bq_sb = wp.tile([128, HPC], F32)
        msc_sb = wp.tile([128, 2, 128], BF)
        msk_sb = msc_sb[:, 0, :]  # 0/1 lower-tri (k<=q) [k,q]
        onk_sb = msc_sb[:, 1, :]  # ones

        for b in range(B):
            s0 = b * S
            x_sb = bp.tile([128, 16, S], BF, tag="x")
            if b == 0:
                nc.sync.dma_start(wq_sb[:], wq3[:].rearrange("p (t d) -> p t d", t=16))
                nc.sync.dma_start(wk_sb[:], wk3[:].rearrange("p (t d) -> p t d", t=16))
                nc.sync.dma_start(wv_sb[:], wv3[:].rearrange("p (t d) -> p t d", t=16))
            for t in range(16):
                nc.sync.dma_start(x_sb[:, t, :], xT[t * 128 : (t + 1) * 128, s0 : s0 + S])
                if b == 0 and t == 1:
                    nc.sync.dma_start(bq_sb[:], bqd[:])
                    nc.sync.dma_start(
                        msc_sb[:], mscd[:].rearrange("p (a k) -> p a k", a=2)
                    )
                if b == 0 and t == 15:
                    nc.sync.dma_start(
                        wo_sb[:], wo3[:].rearrange("p (h e) -> p h e", h=HPC)
                    )
            qT_sb = bp.tile([128, HPC, S], BF, tag="qT")
            kT_sb = bp.tile([128, HPC, S], BF, tag="kT")
            v_sb = bp.tile([128, 16, DC], BF, tag="v")
            ctxN_sb = bp.tile([128, HPC, S], BF, tag="cN", bufs=2)

            # --- q/k/v projections (contract E in 16 chunks of 128) ---
            # Wave-major: all 8 chains of a j-block accumulate in parallel
            # across the 8 PSUM banks, inner loop over t, so PE advances
            # with the x DMA wave instead of stalling chain-by-chain.
            # k bias is skipped entirely: softmax is invariant to the
            # per-query constant q·bk it would add to every score row.
            for j in range(4):
                js = slice(j * 512, (j + 1) * 512)
                tags = [("pr", 2), ("pr", 2), ("sc", 3), ("sc", 3),
                        ("sc", 3), ("cx", 3), ("cx", 3), ("cx", 3)]
                qk_ps = []
                for m in range(HPC):
                    tg, bf = tags[2 * m], tags[2 * m + 1]
                    qm_ps = ps.tile([128, 512], F32, tag=tg[0], bufs=tg[1], name=f"q{m}")
                    km_ps = ps.tile([128, 512], F32, tag=bf[0], bufs=bf[1], name=f"k{m}")
                    qk_ps.append(qm_ps)
                    qk_ps.append(km_ps)
                v_pss = []
                for si in range(4):
                    vs_ps = ps.tile(
                        [128, DC], F32, tag=tags[4 + si][0], bufs=tags[4 + si][1],
                        name=f"v{si}",
                    )
                    v_pss.append(vs_ps)
                for t in range(16):
                    for m in range(HPC):
                        nc.tensor.matmul(
                            qk_ps[2 * m][:],
                            wq_sb[:, t, m * 128 : (m + 1) * 128],
                            x_sb[:, t, js],
                            start=(t == 0),
                            stop=(t == 15),
                        )
                        nc.tensor.matmul(
                            qk_ps[2 * m + 1][:],
                            wk_sb[:, t, m * 128 : (m + 1) * 128],
                            x_sb[:, t, js],
                            start=(t == 0),
                            stop=(t == 15),
                        )
                    for si in range(4):
                        nc.tensor.matmul(
                            v_pss[si][:],
                            x_sb[:, t, (4 * j + si) * 128 : (4 * j + si + 1) * 128],
                            wv_sb[:, t, :],
                            start=(t == 0),
                            stop=(t == 15),
                        )
                for m in range(HPC):
                    nc.scalar.activation(
                        qT_sb[:, m, js], qk_ps[2 * m][:], AF.Identity,
                        bias=bq_sb[:, m : m + 1],
                    )
                    nc.vector.tensor_copy(kT_sb[:, m, js], qk_ps[2 * m + 1][:])
                for si in range(4):
                    if si == 3:
                        nc.scalar.copy(v_sb[:, 4 * j + si, :], v_pss[si][:])
                    else:
                        eng = [nc.vector, nc.vector, nc.vector][si]
                        eng.tensor_copy(v_sb[:, 4 * j + si, :], v_pss[si][:])

            # --- causal attention, scores kept transposed [k, q] ---
            # Valid-column trim: chunk kc only contributes to queries
            # q >= kc*128, so all score/exp/den/ctx work runs on the
            # [off:512] column slice. qb outer / h inner so the two heads'
            # pipelines hide each other's boundary bubbles.
            for qb in range(4):
                q0 = qb * 512
                kmax = 4 * qb + 4
                for h in range(HPC):
                    hd = slice(h * 128, (h + 1) * 128)
                    pts = []
                    den_ps = ps.tile([128, 512], F32, tag="pr", bufs=2)
                    ctx_ps = ps.tile([128, 512], F32, tag="cx", bufs=3)
                    LAG = 2
                    # interleave den/ctx accumulation (lagging LAG tiles)
                    # between score matmuls so PE never waits on ACT exp
                    for kc in range(kmax + LAG):
                        if kc < kmax:
                            diag = kc - 4 * qb
                            off = max(0, 128 * diag)
                            sc_ps = ps.tile([128, 512], F32, tag="sc", bufs=3)
                            nc.tensor.matmul(
                                sc_ps[:, off:512],
                                kT_sb[:, h, kc * 128 : (kc + 1) * 128],
                                qT_sb[:, h, q0 + off : q0 + 512],
                                start=True,
                                stop=True,
                            )
                            p_t = pp.tile([128, 512], BF)
                            nc.scalar.activation(
                                p_t[:, off:512], sc_ps[:, off:512], AF.Exp
                            )
                            if diag >= 0:
                                nc.vector.tensor_tensor(
                                    p_t[:, off : off + 128],
                                    p_t[:, off : off + 128],
                                    msk_sb,
                                    ALU.mult,
                                )
                            pts.append((p_t, off))
                        j = kc - LAG
                        if j >= 0:
                            pj, oj = pts[j]
                            nc.tensor.matmul(
                                den_ps[:, oj:512],
                                onk_sb,
                                pj[:, oj:512],
                                start=(j == 0),
                                stop=(j == kmax - 1),
                                skip_group_check=True,
                            )
                            nc.tensor.matmul(
                                ctx_ps[:, oj:512],
                                v_sb[:, j, hd],
                                pj[:, oj:512],
                                start=(j == 0),
                                stop=(j == kmax - 1),
                                skip_group_check=True,
                            )
                    recb_sb = dp.tile([128, 512], F32, tag="recb", bufs=2)
                    nc.vector.reciprocal(recb_sb[:], den_ps[:])
                    nc.vector.tensor_tensor(
                        ctxN_sb[:, h, q0 : q0 + 512], ctx_ps[:], recb_sb[:], ALU.mult
                    )

            # --- output projection (contract per-core d=256 in 2 head chunks) ---
            # y_ps rotates across all 3 psum tags; drains rotate across
            # DVE/ACT/Pool so copies never pace the PE.
            ytags = [("pr", 2), ("sc", 3), ("cx", 3)]
            yengs = [nc.vector, nc.scalar, nc.vector]
            for qc in range(16):
                for ep in range(2):
                    y_t = yp.tile([128, 1024], BF)
                    for ei in range(2):
                        eb = ep * 2 + ei
                        n = qc * 4 + eb
                        tg = ytags[n % 3]
                        y_ps = ps.tile([128, 512], F32, tag=tg[0], bufs=tg[1])
                        nc.tensor.matmul(
                            y_ps[:],
                            ctxN_sb[:, 0, qc * 128 : (qc + 1) * 128],
                            wo_sb[:, 0, eb * 512 : (eb + 1) * 512],
                            start=True,
                            stop=False,
                        )
                        nc.tensor.matmul(
                            y_ps[:],
                            ctxN_sb[:, 1, qc * 128 : (qc + 1) * 128],
                            wo_sb[:, 1, eb * 512 : (eb + 1) * 512],
                            start=False,
                            stop=True,
                        )
                        eng = yengs[n % 3]
                        if eng is nc.scalar:
                            eng.copy(y_t[:, ei * 512 : (ei + 1) * 512], y_ps[:])
                        else:
                            eng.tensor_copy(y_t[:, ei * 512 : (ei + 1) * 512], y_ps[:])
                    nc.sync.dma_start(
                        yd[
                            s0 + qc * 128 : s0 + (qc + 1) * 128,
                            ep * 1024 : (ep + 1) * 1024,
                        ],
                        y_t[:],
                    )
    _split_excess_waits(nc)
    return nc


def _pack_w(w):
    # [E, DC] -> [128, 16*DC] so row p holds w[t*128+p, :] for t=0..15
    return np.ascontiguousarray(
        w.reshape(16, 128, DC).transpose(1, 0, 2).reshape(128, 16 * DC)
    )


def kernel(**inputs):
    global LAST_RESULTS, _NC_CACHE
    x = np.asarray(inputs["x"], np.float32)
    Wq = np.asarray(inputs["Wq"], np.float32)
    bq = np.asarray(inputs["bq"], np.float32)
    Wk = np.asarray(inputs["Wk"], np.float32)
    bk = np.asarray(inputs["bk"], np.float32)
    Wv = np.asarray(inputs["Wv"], np.float32)
    bv = np.asarray(inputs["bv"], np.float32)
    Wo = np.asarray(inputs["Wo"], np.float32)
    bo = np.asarray(inputs["bo"], np.float32)

    xT = np.ascontiguousarray(x.reshape(BS, E).T).astype(bf16)
    kk = np.arange(128)[:, None]
    qq = np.arange(128)[None, :]
    misc = np.concatenate(
        [
            np.where(kk <= qq, 1.0, 0.0),
            np.ones((128, 128)),
        ],
        axis=1,
    ).astype(bf16)

    in_maps = []
    for c in range(NCORES):
        dsl = slice(c * DC, (c + 1) * DC)
        in_maps.append(
            {
                "xT": xT,
                "wq3": _pack_w((Wq[dsl].T * SCALE).astype(bf16)),
                "wk3": _pack_w(Wk[dsl].T.astype(bf16)),
                "wv3": _pack_w(Wv[dsl].T.astype(bf16)),
                "wo3": np.ascontiguousarray(
                    Wo[:, dsl].T.astype(bf16)
                    .reshape(HPC, 128, E)
                    .transpose(1, 0, 2)
                    .reshape(128, HPC * E)
                ),
                "bq": np.ascontiguousarray(
                    (bq[dsl] * SCALE).astype(np.float32).reshape(HPC, 128).T
                ),
                "misc": misc,
            }
        )

    if _NC_CACHE is None:
        _NC_CACHE = _build()
    res = run_bass_kernel_spmd(_NC_CACHE, in_maps, core_ids=list(range(NCORES)), trace=TRACE)
    LAST_RESULTS = res

    acc = None
    for r in res.results:
        yc = np.asarray(r["y"], np.float32)
        acc = yc if acc is None else acc + yc
    bo_eff = bo + bv @ Wo.T
    acc += bo_eff[None, :]
    return acc.reshape(B, S, E).astype(np.float32)


# revision 14
# speedup vs baseline: 1.3502x; 1.0054x over previous
import sys

sys.path.insert(0, "/opt/trn_rl_repo")

from contextlib import ExitStack

import ml_dtypes
import numpy as np

from concourse import bass, mybir, tile
from concourse.bass_utils import run_bass_kernel_spmd
from concourse.vector_clock import ScopedClock


def _patched_drain_and_barrier(self, tick_clock, wait_clock):
    # Workaround: this compiler rejects a drain carrying >1 sem wait
    # ([NCC_INLA001]); split extra waits onto single-wait nops.
    drain_inst = self.nc.sync.drain()
    wait_clock.add_sem_waits(
        drain_inst.ins, ScopedClock({None: tick_clock.global_clock})
    )
    si = drain_inst.ins.sync_info
    waits = list(si.on_wait) if si and si.on_wait else []
    if len(waits) > 1:
        drain_inst.ins.sync_info = mybir.SyncInfo(
            on_wait=[waits[0]], on_update=list(si.on_update or [])
        )
        for w in waits[1:]:
            nop = self.nc.sync.nop(nofuse=True)
            nop.ins.sync_info = mybir.SyncInfo(on_wait=[w], on_update=[])
    self.nc.all_engine_barrier()
    popped = self.nc._tile_sem_poison_stack.pop()
    assert popped is self._sem_poison
    self.nc.clear_and_free_semaphores(list(self.sems.allocated().values()))
    self.nc.all_engine_barrier()


tile.TileContext._drain_and_barrier = _patched_drain_and_barrier


def _split_excess_waits(nc, limit=1):
    # Workaround: this compiler allows only one sem wait on several
    # instruction encodings; move extra waits onto same-engine nops.
    eng_map = {
        mybir.EngineType.PE: nc.tensor,
        mybir.EngineType.Activation: nc.scalar,
        mybir.EngineType.DVE: nc.vector,
        mybir.EngineType.Pool: nc.gpsimd,
        mybir.EngineType.SP: nc.sync,
    }
    for blk in nc.cur_f.blocks:
        orig = list(blk.instructions)
        out = []
        for ins in orig:
            si = ins.sync_info
            waits = list(si.on_wait) if si and si.on_wait else []
            eng = eng_map.get(ins.engine)
            if len(waits) > limit and eng is not None:
                extra, keep = waits[:-limit], waits[-limit:]
                for w in extra:
                    nop = eng.nop(nofuse=True).ins
                    tail = nc.cur_f.blocks[-1].instructions
                    assert tail[-1] is nop
                    tail.pop()
                    nop.sync_info = mybir.SyncInfo(on_wait=[w], on_update=[])
                    out.append(nop)
                ins.sync_info = mybir.SyncInfo(
                    on_wait=keep, on_update=list(si.on_update or [])
                )
            out.append(ins)
        blk.instructions[:] = out

bf16 = ml_dtypes.bfloat16
BF = bass.mybir.dt.bfloat16
F32 = bass.mybir.dt.float32
AF = mybir.ActivationFunctionType
ALU = mybir.AluOpType

B, S, E, H, D = 2, 2048, 2048, 16, 128
BS = B * S
NCORES = 8
HPC = H // NCORES  # heads per core
DC = HPC * D  # per-core head-dim width (256)
SCALE = 1.0 / float(np.sqrt(D))

TRACE = False
LAST_RESULTS = None
_NC_CACHE = None


def _build():
    nc = bass.Bass()
    xT = nc.declare_dram_parameter("xT", (E, BS), BF, isOutput=False)
    # weights pre-packed on host to [128, ...] so each is ONE contiguous DMA
    wq3 = nc.declare_dram_parameter("wq3", (128, 16 * DC), BF, isOutput=False)
    wk3 = nc.declare_dram_parameter("wk3", (128, 16 * DC), BF, isOutput=False)
    wv3 = nc.declare_dram_parameter("wv3", (128, 16 * DC), BF, isOutput=False)
    wo3 = nc.declare_dram_parameter("wo3", (128, HPC * E), BF, isOutput=False)
    bqd = nc.declare_dram_parameter("bq", (128, HPC), F32, isOutput=False)
    mscd = nc.declare_dram_parameter("misc", (128, 2 * 128), BF, isOutput=False)
    yd = nc.declare_dram_parameter("y", (BS, E), BF, isOutput=True)

    with ExitStack() as ctx:
        tc = ctx.enter_context(tile.TileContext(nc))
        wp = ctx.enter_context(tc.tile_pool(name="wp", bufs=1))
        bp = ctx.enter_context(tc.tile_pool(name="bp", bufs=1))
        pp = ctx.enter_context(tc.tile_pool(name="pp", bufs=8))
        dp = ctx.enter_context(tc.tile_pool(name="dp", bufs=2))
        yp = ctx.enter_context(tc.tile_pool(name="yp", bufs=6))
        ps = ctx.enter_context(tc.tile_pool(name="ps", bufs=1, space="PSUM"))

        wq_sb = wp.tile([128, 16, DC], BF)
        wk_sb = wp.tile([128, 16, DC], BF)
        wv_sb = wp.tile([128, 16, DC], BF)
        wo_sb = wp.tile([128, HPC, E], BF)
        bq_sb = wp.tile([128, HPC], F32)
        msc_sb = wp.tile([128, 2, 128], BF)
        msk_sb = msc_sb[:, 0, :]  # 0/1 lower-tri (k<=q) [k,q]
        onk_sb = msc_sb[:, 1, :]  # ones

        for b in range(B):
            s0 = b * S
            x_sb = bp.tile([128, 16, S], BF, tag="x")
            if b == 0:
                # 4 t-chunks per weight: the first j-block's t=0 matmuls
                # only wait on a 256KB chunk, not the whole 1MB tensor
                for tc4 in range(4):
                    cs = slice(tc4 * 4 * DC, (tc4 + 1) * 4 * DC)
                    nc.sync.dma_start(
                        wq_sb[:, tc4 * 4 : (tc4 + 1) * 4, :],
                        wq3[:, cs].rearrange("p (t d) -> p t d", t=4),
                    )
                    nc.sync.dma_start(
                        wk_sb[:, tc4 * 4 : (tc4 + 1) * 4, :],
                        wk3[:, cs].rearrange("p (t d) -> p t d", t=4),
                    )
                    nc.sync.dma_start(
                        wv_sb[:, tc4 * 4 : (tc4 + 1) * 4, :],
                        wv3[:, cs].rearrange("p (t d) -> p t d", t=4),
                    )
            for t in range(16):
                nc.sync.dma_start(x_sb[:, t, :], xT[t * 128 : (t + 1) * 128, s0 : s0 + S])
                if b == 0 and t == 1:
                    nc.sync.dma_start(bq_sb[:], bqd[:])
                    nc.sync.dma_start(
                        msc_sb[:], mscd[:].rearrange("p (a k) -> p a k", a=2)
                    )
                if b == 0 and t == 15:
                    nc.sync.dma_start(
                        wo_sb[:], wo3[:].rearrange("p (h e) -> p h e", h=HPC)
                    )
            qT_sb = bp.tile([128, HPC, S], BF, tag="qT")
            kT_sb = bp.tile([128, HPC, S], BF, tag="kT")
            v_sb = bp.tile([128, 16, DC], BF, tag="v")
            ctxN_sb = bp.tile([128, HPC, S], BF, tag="cN", bufs=2)

            # --- q/k/v projections (contract E in 16 chunks of 128) ---
            # Wave-major: all 8 chains of a j-block accumulate in parallel
            # across the 8 PSUM banks, inner loop over t, so PE advances
            # with the x DMA wave instead of stalling chain-by-chain.
            # k bias is skipped entirely: softmax is invariant to the
            # per-query constant q·bk it would add to every score row.
            for j in range(4):
                js = slice(j * 512, (j + 1) * 512)
                tags = [("pr", 2), ("pr", 2), ("sc", 3), ("sc", 3),
                        ("sc", 3), ("cx", 3), ("cx", 3), ("cx", 3)]
                qk_ps = []
                for m in range(HPC):
                    tg, bf = tags[2 * m], tags[2 * m + 1]
                    qm_ps = ps.tile([128, 512], F32, tag=tg[0], bufs=tg[1], name=f"q{m}")
                    km_ps = ps.tile([128, 512], F32, tag=bf[0], bufs=bf[1], name=f"k{m}")
                    qk_ps.append(qm_ps)
                    qk_ps.append(km_ps)
                v_pss = []
                for si in range(4):
                    vs_ps = ps.tile(
                        [128, DC], F32, tag=tags[4 + si][0], bufs=tags[4 + si][1],
                        name=f"v{si}",
                    )
                    v_pss.append(vs_ps)
                for t in range(16):
                    for m in range(HPC):
                        nc.tensor.matmul(
                            qk_ps[2 * m][:],
                            wq_sb[:, t, m * 128 : (m + 1) * 128],
                            x_sb[:, t, js],
                            start=(t == 0),
                            stop=(t == 15),
                        )
                        nc.tensor.matmul(
                            qk_ps[2 * m + 1][:],
                            wk_sb[:, t, m * 128 : (m + 1) * 128],
                            x_sb[:, t, js],
                            start=(t == 0),
                            stop=(t == 15),
                        )
                    for si in range(4):
                        nc.tensor.matmul(
                            v_pss[si][:],
                            x_sb[:, t, (4 * j + si) * 128 : (4 * j + si + 1) * 128],
                            wv_sb[:, t, :],
                            start=(t == 0),
                            stop=(t == 15),
                        )
                for m in range(HPC):
                    nc.scalar.activation(
                        qT_sb[:, m, js], qk_ps[2 * m][:], AF.Identity,
                        bias=bq_sb[:, m : m + 1],
                    )
                    nc.vector.tensor_copy(kT_sb[:, m, js], qk_ps[2 * m + 1][:])
                for si in range(4):
                    if si == 3:
                        nc.scalar.copy(v_sb[:, 4 * j + si, :], v_pss[si][:])
                    else:
                        eng = [nc.vector, nc.vector, nc.vector][si]
                        eng.tensor_copy(v_sb[:, 4 * j + si, :], v_pss[si][:])

            # --- causal attention, scores kept transposed [k, q] ---
            # Valid-column trim: chunk kc only contributes to queries
            # q >= kc*128, so all score/exp/den/ctx work runs on the
            # [off:512] column slice. qb outer / h inner so the two heads'
            # pipelines hide each other's boundary bubbles.
            for qb in range(4):
                q0 = qb * 512
                kmax = 4 * qb + 4
                for h in range(HPC):
                    hd = slice(h * 128, (h + 1) * 128)
                    pts = []
                    den_ps = ps.tile([128, 512], F32, tag="pr", bufs=2)
                    ctx_ps = ps.tile([128, 512], F32, tag="cx", bufs=3)
                    LAG = 2
                    # interleave den/ctx accumulation (lagging LAG tiles)
                    # between score matmuls so PE never waits on ACT exp
                    for kc in range(kmax + LAG):
                        if kc < kmax:
                            diag = kc - 4 * qb
                            off = max(0, 128 * diag)
                            sc_ps = ps.tile([128, 512], F32, tag="sc", bufs=3)
                            nc.tensor.matmul(
                                sc_ps[:, off:512],
                                kT_sb[:, h, kc * 128 : (kc + 1) * 128],
                                qT_sb[:, h, q0 + off : q0 + 512],
                                start=True,
                                stop=True,
                            )
                            p_t = pp.tile([128, 512], BF)
                            nc.scalar.activation(
                                p_t[:, off:512], sc_ps[:, off:512], AF.Exp
                            )
                            if diag >= 0:
                                nc.vector.tensor_tensor(
                                    p_t[:, off : off + 128],
                                    p_t[:, off : off + 128],
                                    msk_sb,
                                    ALU.mult,
                                )
                            pts.append((p_t, off))
                        j = kc - LAG
                        if j >= 0:
                            pj, oj = pts[j]
                            nc.tensor.matmul(
                                den_ps[:, oj:512],
                                onk_sb,
                                pj[:, oj:512],
                                start=(j == 0),
                                stop=(j == kmax - 1),
                                skip_group_check=True,
                            )
                            nc.tensor.matmul(
                                ctx_ps[:, oj:512],
                                v_sb[:, j, hd],
                                pj[:, oj:512],
                                start=(j == 0),
                                stop=(j == kmax - 1),
                                skip_group_check=True,
                            )
                    recb_sb = dp.tile([128, 512], F32, tag="recb", bufs=2)
                    nc.vector.reciprocal(recb_sb[:], den_ps[:])
                    nc.vector.tensor_tensor(
                        ctxN_sb[:, h, q0 : q0 + 512], ctx_ps[:], recb_sb[:], ALU.mult
                    )

            # --- output projection (contract per-core d=256 in 2 head chunks) ---
            # y_ps rotates across all 3 psum tags; drains rotate across
            # DVE/ACT/Pool so copies never pace the PE.
            ytags = [("pr", 2), ("sc", 3), ("cx", 3)]
            yengs = [nc.vector, nc.scalar, nc.vector, nc.scalar, nc.vector, nc.scalar]
            for qc in range(16):
                for ep in range(2):
                    y_t = yp.tile([128, 1024], BF)
                    for ei in range(2):
                        eb = ep * 2 + ei
                        n = qc * 4 + eb
                        tg = ytags[n % 3]
                        y_ps = ps.tile([128, 512], F32, tag=tg[0], bufs=tg[1])
                        nc.tensor.matmul(
                            y_ps[:],
                            ctxN_sb[:, 0, qc * 128 : (qc + 1) * 128],
                            wo_sb[:, 0, eb * 512 : (eb + 1) * 512],
                            start=True,
                            stop=False,
                        )
                        nc.tensor.matmul(
                            y_ps[:],
                            ctxN_sb[:, 1, qc * 128 : (qc + 1) * 128],
                            wo_sb[:, 1, eb * 512 : (eb + 1) * 512],
                            start=False,
                            stop=True,
                        )
                        eng = yengs[n % 2]
                        if eng is nc.scalar:
                            eng.copy(y_t[:, ei * 512 : (ei + 1) * 512], y_ps[:])
                        else:
                            eng.tensor_copy(y_t[:, ei * 512 : (ei + 1) * 512], y_ps[:])
                    nc.sync.dma_start(
                        yd[
                            s0 + qc * 128 : s0 + (qc + 1) * 128,
                            ep * 1024 : (ep + 1) * 1024,
                        ],
                        y_t[:],
                    )
    _split_excess_waits(nc)
    return nc


def _pack_w(w):
    # [E, DC] -> [128, 16*DC] so row p holds w[t*128+p, :] for t=0..15
    return np.ascontiguousarray(
        w.reshape(16, 128, DC).transpose(1, 0, 2).reshape(128, 16 * DC)
    )


def kernel(**inputs):
    global LAST_RESULTS, _NC_CACHE
    x = np.asarray(inputs["x"], np.float32)
    Wq = np.asarray(inputs["Wq"], np.float32)
    bq = np.asarray(inputs["bq"], np.float32)
    Wk = np.asarray(inputs["Wk"], np.float32)
    bk = np.asarray(inputs["bk"], np.float32)
    Wv = np.asarray(inputs["Wv"], np.float32)
    bv = np.asarray(inputs["bv"], np.float32)
    Wo = np.asarray(inputs["Wo"], np.float32)
    bo = np.asarray(inputs["bo"], np.float32)

    xT = np.ascontiguousarray(x.reshape(BS, E).T).astype(bf16)
    kk = np.arange(128)[:, None]
    qq = np.arange(128)[None, :]
    misc = np.concatenate(
        [
            np.where(kk <= qq, 1.0, 0.0),
            np.ones((128, 128)),
        ],
        axis=1,
    ).astype(bf16)

    in_maps = []
    for c in range(NCORES):
        dsl = slice(c * DC, (c + 1) * DC)
        in_maps.append(
            {
                "xT": xT,
                "wq3": _pack_w((Wq[dsl].T * SCALE).astype(bf16)),
                "wk3": _pack_w(Wk[dsl].T.astype(bf16)),
                "wv3": _pack_w(Wv[dsl].T.astype(bf16)),
                "wo3": np.ascontiguousarray(
                    Wo[:, dsl].T.astype(bf16)
                    .reshape(HPC, 128, E)
                    .transpose(1, 0, 2)
                    .reshape(128, HPC * E)
                ),
                "bq": np.ascontiguousarray(
                    (bq[dsl] * SCALE).astype(np.float32).reshape(HPC, 128).T
                ),
                "misc": misc,
            }
        )

    if _NC_CACHE is None:
        _NC_CACHE = _build()
    res = run_bass_kernel_spmd(_NC_CACHE, in_maps, core_ids=list(range(NCORES)), trace=TRACE)
    LAST_RESULTS = res

    acc = None
    for r in res.results:
        yc = np.asarray(r["y"], np.float32)
        acc = yc if acc is None else acc + yc
    bo_eff = bo + bv @ Wo.T
    acc += bo_eff[None, :]
    return acc.reshape(B, S, E).astype(np.float32)
